# revision 41
# baseline (speedup 1.0000x reference)
"""Trainium2 Bass kernel for GAT(3 layers, 4 heads) + JK-LSTM + global pool + MLP.

Sharding: nodes (and their incoming edges) are partitioned across 8 NeuronCores.
Layer-0 node transform is computed replicated (input x is replicated); layers 1-2
exchange activations via AllGather (bf16). Segment softmax + scatter-add
aggregation is done with one-hot matmuls over destination-sorted edge chunks.
The JK-LSTM (bf16 weights resident in SBUF), attention, pooling and MLP are
data-parallel over the node shard, with a final AllReduce for the graph pooling.
"""
import os
import sys

for _p in ("/opt/trn_rl_repo", "/root/.axon_site/_ro/trn_rl_repo"):
    if os.path.isdir(_p) and _p not in sys.path:
        sys.path.append(_p)

import numpy as np

import concourse.bass as bass
import concourse.bacc as bacc
import concourse.mybir as mybir
import concourse.tile as tile

P = 128
N, E, G = 10000, 160000, 64
IN_C, HID, HEADS, OUT = 128, 128, 4, 8
C, L, HL = 512, 3, 768
NCORES = 8
NPC = N // NCORES          # 1250 nodes per core
NPCP = 1280                # padded (10 tiles of 128)
NTILES_OWN = NPCP // P     # 10
NT = NPCP * NCORES         # 10240 padded total
NTILES_ALL = NT // P       # 80
DH = 520                   # haug row: h(512) | a_s(4) | pad(4)
SENT = 255.0               # sentinel dest-local for padding edges

f32 = mybir.dt.float32
bf16 = mybir.dt.bfloat16
f32r = mybir.dt.float32r
i32 = mybir.dt.int32
AF = mybir.ActivationFunctionType
ALU = mybir.AluOpType

_CACHE = {}


def _gp(n):
    """node id -> padded global slot (half-major so AllGather halves are
    contiguous: [half][core][640])"""
    n = np.asarray(n)
    k = n // NPC
    s = n % NPC
    half = (s >= NPCP // 2).astype(n.dtype) if hasattr(s, "astype") else int(s >= NPCP // 2)
    return half * (NCORES * (NPCP // 2)) + k * (NPCP // 2) + (s - half * (NPCP // 2))


def build_tables(edge_index, batch):
    """Host-side preprocessing: destination-sorted, per-(core,tile) chunked edge
    tables, pooling matrix."""
    ei = np.concatenate(
        [np.asarray(edge_index), np.tile(np.arange(N, dtype=np.int32), (2, 1))], axis=1
    )
    s_arr, d_arr = ei[0].astype(np.int64), ei[1].astype(np.int64)
    own = d_arr // NPC
    per_ct = {}
    for k in range(NCORES):
        m = own == k
        sk, dk = s_arr[m], d_arr[m]
        dloc = dk - NPC * k
        t_all = dloc // P
        for t in range(NTILES_OWN):
            tm = t_all == t
            per_ct[(k, t)] = (sk[tm], dloc[tm] - t * P)
    nct = max((len(v[0]) + P - 1) // P for v in per_ct.values())
    srcidx = np.zeros((NCORES, P, NTILES_OWN * nct), np.int32)
    dlt = np.full((NCORES, P, NTILES_OWN * nct), SENT, np.float32)
    for k in range(NCORES):
        for t in range(NTILES_OWN):
            sk, dloc = per_ct[(k, t)]
            ne = len(sk)
            col0 = t * nct
            for c in range((ne + P - 1) // P):
                lo, hi = c * P, min((c + 1) * P, ne)
                srcidx[k, 0 : hi - lo, col0 + c] = _gp(sk[lo:hi])
                dlt[k, 0 : hi - lo, col0 + c] = dloc[lo:hi]
    # precomputed one-hot scatter masks, both orientations
    ncols = NTILES_OWN * nct
    import ml_dtypes
    ar = np.arange(P, dtype=np.float32)
    stall = np.zeros((NCORES, ncols, P, P), ml_dtypes.bfloat16)
    sall = np.zeros((NCORES, ncols, P, P), ml_dtypes.bfloat16)
    for k in range(NCORES):
        stk = (dlt[k].T[:, :, None] == ar[None, None, :]).astype(ml_dtypes.bfloat16)  # [ncols, e, n]
        stall[k] = stk
        sall[k] = stk.transpose(0, 2, 1)
    batch = np.asarray(batch)
    cnt = np.maximum(np.bincount(batch, minlength=G), 1).astype(np.float32)
    poolmat = np.zeros((NCORES, P, NTILES_OWN, G), np.float32)
    inv = 1.0 / cnt
    for n in range(N):
        k, sl = n // NPC, n % NPC
        poolmat[k, sl % P, sl // P, batch[n]] = inv[batch[n]]
    return nct, srcidx, dlt, poolmat, stall, sall


def build_nc(nct, skip=frozenset(), nswdge=1):
    NCH = NTILES_OWN * nct
    nc = bacc.Bacc("TRN2", target_bir_lowering=False, debug=False, num_devices=NCORES,
                   num_swdge_queues=nswdge)

    # ---------------- kernel I/O ----------------
    d_xT = nc.dram_tensor("xT", [P, NT], f32r, kind="ExternalInput")
    d_W0a = nc.dram_tensor("W0a", [P, 4], f32r, kind="ExternalInput")
    d_adtab0 = nc.dram_tensor("adtab0", [P, 4 * NTILES_OWN], bf16, kind="ExternalInput")
    d_Wad = [None,
             nc.dram_tensor("Wad1", [P, 4, 8], bf16, kind="ExternalInput"),
             nc.dram_tensor("Wad2", [P, 4, 8], bf16, kind="ExternalInput")]
    d_W = [
        nc.dram_tensor("W0d", [P, C], f32r, kind="ExternalInput"),
        nc.dram_tensor("W1d", [C, C], bf16, kind="ExternalInput"),
        nc.dram_tensor("W2d", [C, C], bf16, kind="ExternalInput"),
    ]
    d_brep = [nc.dram_tensor(f"brep{l}", [P, C], f32, kind="ExternalInput") for l in range(L)]
    d_srcidx = nc.dram_tensor("srcidx", [P, NCH], i32, kind="ExternalInput")
    d_dlt = nc.dram_tensor("dlt", [P, NCH], f32, kind="ExternalInput")
    d_stall = nc.dram_tensor("stall", [NCH, P, P], bf16, kind="ExternalInput")
    d_sall = nc.dram_tensor("sall", [NCH, P, P], bf16, kind="ExternalInput")
    d_wih = [nc.dram_tensor(f"WihT_{d}", [C, 4 * HL], bf16, kind="ExternalInput") for d in "fr"]
    d_whh = [nc.dram_tensor(f"WhhT_{d}", [HL, 4 * HL], bf16, kind="ExternalInput") for d in "fr"]
    d_bsum = nc.dram_tensor("bsum", [P, 48], f32, kind="ExternalInput")
    d_attw = nc.dram_tensor("attw", [P, 12], bf16, kind="ExternalInput")
    d_poolmat = nc.dram_tensor("poolmat", [P, NTILES_OWN, G], f32r, kind="ExternalInput")
    d_fc1 = nc.dram_tensor("fc1W", [C, C], f32r, kind="ExternalInput")
    d_fc2 = nc.dram_tensor("fc2W", [C, C], f32r, kind="ExternalInput")
    d_fc3 = nc.dram_tensor("fc3W", [C, OUT], f32r, kind="ExternalInput")
    d_fcb = nc.dram_tensor("fcb", [P, 8], f32, kind="ExternalInput")  # fc1_b | fc2_b
    d_fc3b = nc.dram_tensor("fc3b", [OUT, 1], f32, kind="ExternalInput")
    d_out = nc.dram_tensor("out_T", [OUT, G], f32, kind="ExternalOutput")

    # ---------------- internal DRAM ----------------
    d_haug = [
        nc.dram_tensor("haug0", [NT, DH], bf16),
        nc.dram_tensor("haug1", [NT, DH], bf16, addr_space="Shared"),
        nc.dram_tensor("haug2", [NT, DH], bf16, addr_space="Shared"),
    ]
    d_hsh = [None, nc.dram_tensor("hsh1", [NPCP, DH], bf16), nc.dram_tensor("hsh2", [NPCP, DH], bf16)]
    d_x = [nc.dram_tensor(f"x_l{l}", [NPCP, C], bf16) for l in range(L)]
    d_xt = [nc.dram_tensor(f"xt_l{l}", [C, NPCP], bf16) for l in range(L)]
    d_scores = nc.dram_tensor("scoresd", [6, NPCP], f32)
    d_poolin = nc.dram_tensor("poolin", [G, C], f32)
    d_pooled = nc.dram_tensor("pooled", [G, C], f32, addr_space="Shared")

    RG = [list(range(NCORES))]
    BLKS = [(0, 512), (512, 512), (1024, 256)]  # node blocks of NPCP

    with tile.TileContext(nc) as tc, \
         nc.allow_low_precision(reason="bf16 activations within tolerance"):
        with tc.tile_pool(name="const", bufs=1) as const, \
             tc.tile_pool(name="psum", bufs=1, space="PSUM") as psum, \
             tc.tile_pool(name="lstmp", bufs=1) as lstmp, \
             tc.tile_pool(name="lstmx", bufs=2) as lstmx:
            ident_f = const.tile([P, P], f32)
            from concourse.masks import make_identity

            make_identity(nc, ident_f[:])
            ident = const.tile([P, P], f32r)
            nc.vector.tensor_copy(out=ident[:], in_=ident_f[:])
            identb = const.tile([P, P], bf16)
            nc.vector.tensor_copy(out=identb[:], in_=ident_f[:])
            srcidx = const.tile([P, NCH], i32)
            nc.sync.dma_start(out=srcidx[:], in_=d_srcidx[:, :])
            adtab = [const.tile([P, 4 * NTILES_OWN], bf16, tag=f"adtab{_l}", name=f"adtab{_l}") for _l in range(L)]
            bsum = const.tile([P, 48], f32)
            nc.sync.dma_start(out=bsum[:], in_=d_bsum[:, :])
            attw = const.tile([P, 12], bf16)
            nc.sync.dma_start(out=attw[:], in_=d_attw[:, :])

            # ---------- LSTM state (persistent across sections) ----------
            lstm_state = {}

            def emit_lstm_block(dire, step, b, wih=None, whh=None, wih_dram=None,
                                xtt_reuse=None):
                """Emit one node-block of one LSTM step. Blocks are pointwise-
                independent so they can interleave with GAT tile processing."""
                t = step if dire == 0 else 2 - step
                b0, bw = BLKS[b]
                st_ = lstm_state.setdefault(dire, {"h": [None] * 6})
                if step == 0 and b == 0:
                    st_["c"] = [lstmp.tile([P, NPCP], bf16, tag=f"c{dire}{j}", name=f"c{dire}{j}")
                                for j in range(6)]
                cst = st_["c"]
                if b == 0:
                    st_["h"] = st_.get("hn", [None] * 6)
                    st_["hn"] = [lstmp.tile([P, NPCP], bf16, tag=f"h{j}", bufs=2, name=f"h{j}")
                                 for j in range(6)]
                    st_["sc"] = lstmp.tile([1, NPCP], f32, tag=f"sc{dire}", name=f"sc{dire}")
                    nc.vector.memset(st_["sc"][:], 0.0)
                    if xtt_reuse is not None:
                        st_["xtt"] = xtt_reuse
                    else:
                        st_["xtt"] = lstmx.tile([P, 4, NPCP], bf16, tag="xtt", bufs=2, name="xtt")
                h_prev, hn, sc_acc, xtt = st_["h"], st_["hn"], st_["sc"], st_["xtt"]
                if xtt_reuse is None:
                    nc.sync.dma_start(
                        out=xtt[:, :, b0 : b0 + bw],
                        in_=d_xt[t][:, b0 : b0 + bw].rearrange("(k p) n -> p k n", p=P))
                gates = (0, 2, 3) if step == 0 else (0, 1, 2, 3)
                for j in range(6):
                    gas = {}
                    for gate in gates:
                        gt_row = gate * 6 + j
                        if wih_dram is not None:
                            wt = lstmx.tile([P, 4, P], bf16, tag="wihs", bufs=6, name="wihs")
                            nc.sync.dma_start(
                                out=wt[:],
                                in_=wih_dram[:, gt_row * P : (gt_row + 1) * P]
                                .rearrange("(k p) g -> p k g", p=P))
                            wslc = lambda kc: wt[:, kc, :]
                        else:
                            wslc = lambda kc: wih[:, kc, gt_row * P : (gt_row + 1) * P]
                        ga = lstmp.tile([P, 512], bf16, tag=f"ga{gate}", bufs=2, name=f"ga{gate}")
                        gps = psum.tile([P, 512], f32, tag="gps", bufs=2, name="gps")
                        for kc in range(4):
                            nc.tensor.matmul(
                                out=gps[:, 0:bw], lhsT=wslc(kc),
                                rhs=xtt[:, kc, b0 : b0 + bw],
                                start=(kc == 0), stop=(kc == 3 and step == 0))
                        if step > 0:
                            for kc in range(6):
                                nc.tensor.matmul(
                                    out=gps[:, 0:bw],
                                    lhsT=whh[:, kc, gt_row * P : (gt_row + 1) * P],
                                    rhs=h_prev[kc][:, b0 : b0 + bw],
                                    start=False, stop=(kc == 5))
                        nc.scalar.activation(
                            out=ga[:, 0:bw], in_=gps[:, 0:bw],
                            func=(AF.Tanh if gate == 2 else AF.Sigmoid),
                            bias=bsum[:, dire * 24 + gt_row : dire * 24 + gt_row + 1])
                        gas[gate] = ga
                    csl = cst[j][:, b0 : b0 + bw]
                    if step == 0:
                        nc.vector.tensor_tensor(out=csl, in0=gas[0][:, 0:bw],
                                                in1=gas[2][:, 0:bw], op=ALU.mult)
                    else:
                        nc.vector.tensor_tensor(out=gas[0][:, 0:bw], in0=gas[0][:, 0:bw],
                                                in1=gas[2][:, 0:bw], op=ALU.mult)
                        nc.vector.tensor_tensor(out=csl, in0=csl, in1=gas[1][:, 0:bw], op=ALU.mult)
                        nc.vector.tensor_tensor(out=csl, in0=csl, in1=gas[0][:, 0:bw], op=ALU.add)
                    tnh = gas[2]
                    nc.scalar.activation(out=tnh[:, 0:bw], in_=csl, func=AF.Tanh)
                    nc.vector.tensor_tensor(out=hn[j][:, b0 : b0 + bw], in0=tnh[:, 0:bw],
                                            in1=gas[3][:, 0:bw], op=ALU.mult)
                    scp = psum.tile([1, 512], f32, tag="scp", bufs=1, name="scp")
                    nc.tensor.matmul(
                        out=scp[:, 0:bw],
                        lhsT=attw[:, dire * 6 + j : dire * 6 + j + 1],
                        rhs=hn[j][:, b0 : b0 + bw], start=True, stop=True)
                    nc.vector.tensor_tensor(
                        out=sc_acc[0:1, b0 : b0 + bw], in0=sc_acc[0:1, b0 : b0 + bw],
                        in1=scp[:, 0:bw], op=ALU.add)
                nc.sync.dma_start(out=d_scores[dire * 3 + t, b0 : b0 + bw][None, :],
                                  in_=sc_acc[0:1, b0 : b0 + bw])

            # ============ forward-direction LSTM weights (resident) ============
            with tc.tile_pool(name="lstmwf", bufs=1) as wfp:
                wihf = wfp.tile([P, 4, 4 * HL], bf16, tag="wihf", name="wihf")
                nc.sync.dma_start(out=wihf[:], in_=d_wih[0].rearrange("(k p) g -> p k g", p=P))
                whhf = wfp.tile([P, 6, 4 * HL], bf16, tag="whhf", name="whhf")
                nc.sync.dma_start(out=whhf[:], in_=d_whh[0].rearrange("(k p) g -> p k g", p=P))

                # ================= stage A0 =================
                with tc.tile_pool(name="a0", bufs=2) as a0p:
                    W0t = a0p.tile([P, C], f32r, tag="w0", bufs=1)
                    nc.sync.dma_start(out=W0t[:], in_=d_W[0][:, :])
                    W0a = a0p.tile([P, 4], f32r, tag="w0a", bufs=1)
                    nc.sync.dma_start(out=W0a[:], in_=d_W0a[:, :])
                    nc.sync.dma_start(out=adtab[0][:], in_=d_adtab0[:, :])
                    for nt in range(NTILES_ALL):
                        xt_t = a0p.tile([P, P], f32r, tag="xt", bufs=4)
                        nc.sync.dma_start(out=xt_t[:], in_=d_xT[:, nt * P : (nt + 1) * P])
                        ps = psum.tile([P, C], f32, tag="ade", bufs=2, name="psa0")
                        nc.tensor.matmul(out=ps[:], lhsT=xt_t[:], rhs=W0t[:], start=True, stop=True)
                        ps8 = psum.tile([P, 8], f32, tag="pso", bufs=2, name="psa0a")
                        nc.tensor.matmul(out=ps8[:, 0:4], lhsT=xt_t[:], rhs=W0a[:], start=True, stop=True)
                        ht = a0p.tile([P, DH], bf16, tag="ht", bufs=4)
                        nc.scalar.copy(out=ht[:, 0:C], in_=ps[:])
                        nc.scalar.copy(out=ht[:, C : C + 4], in_=ps8[:, 0:4])
                        nc.sync.dma_start(out=d_haug[0][nt * P : (nt + 1) * P, :], in_=ht[:])

                # ======== GAT layers (+ interleaved fwd-LSTM steps) ========
                KGRP = 4
                for l in range(L):
                    with tc.tile_pool(name=f"b{l}", bufs=2) as bp, \
                         tc.tile_pool(name=f"b{l}g", bufs=10) as bg:
                        brep = bp.tile([P, C], f32, tag="brep", bufs=1)
                        nc.sync.dma_start(out=brep[:], in_=d_brep[l][:, :])
                        if l < L - 1:
                            Wn = bp.tile([P, 4, C], bf16, tag="wn", bufs=1)
                            for kc in range(4):
                                nc.sync.dma_start(out=Wn[:, kc, :], in_=d_W[l + 1][kc * P : (kc + 1) * P, :])
                            Wadn = bp.tile([P, 4, 8], bf16, tag="wadn", bufs=1)
                            nc.sync.dma_start(out=Wadn[:], in_=d_Wad[l + 1][:, :, :])
                        # hide the inbound AllGather: run the previous layer's
                        # LSTM step (blocks 0-1) before this layer's gathers
                        if l >= 1 and "lstm" not in skip:
                            emit_lstm_block(0, l - 1, 0, wih=wihf, whh=whhf)
                            emit_lstm_block(0, l - 1, 1, wih=wihf, whh=whhf)
                        # scheduler hint: this layer's gather-dependent work can't
                        # start until the inbound AllGather lands; keep it from
                        # being queued ahead of ready LSTM work.
                        LWAIT = {0: 0.0, 1: 0.45, 2: 0.85}
                        for j in range(NTILES_OWN):
                            tc.tile_set_cur_wait(LWAIT[l], enable=(l >= 1))
                            ps_out = psum.tile([P, C], f32, tag="pso", bufs=2, name="ps_out")
                            ps_den = psum.tile([P, C], f32, tag="psd", bufs=1, name="ps_den")
                            for g0 in range(0, nct, KGRP):
                                gw = min(KGRP, nct - g0)
                                colg = j * nct + g0
                                hgs = []
                                st4 = bp.tile([P, KGRP, P], bf16, tag="st4", bufs=3, name="st4")
                                nc.sync.dma_start(
                                    out=st4[:, 0:gw, :],
                                    in_=d_stall[colg : colg + gw].rearrange("c e n -> e c n"))
                                s4 = bp.tile([P, KGRP, P], bf16, tag="s4", bufs=3, name="s4")
                                nc.sync.dma_start(
                                    out=s4[:, 0:gw, :],
                                    in_=d_sall[colg : colg + gw].rearrange("c n e -> n c e"))
                                ade = psum.tile([P, C], f32, tag="ade", bufs=2, name="ade")
                                for ci in range(gw):
                                    col = colg + ci
                                    hg = bg.tile([P, DH], bf16, tag="hg", name="hg")
                                    nc.gpsimd.indirect_dma_start(
                                        out=hg[:], out_offset=None,
                                        in_=d_haug[0 if "ag" in skip else l][:, :],
                                        in_offset=bass.IndirectOffsetOnAxis(ap=srcidx[:, col : col + 1], axis=0))
                                    nc.tensor.matmul(out=ade[:, ci * 4 : ci * 4 + 4], lhsT=s4[:, ci, :],
                                                     rhs=adtab[l][:, j * 4 : (j + 1) * 4],
                                                     start=True, stop=False, skip_group_check=True)
                                    nc.tensor.matmul(out=ade[:, ci * 4 : ci * 4 + 4], lhsT=identb[:, :],
                                                     rhs=hg[:, C : C + 4],
                                                     start=False, stop=True, skip_group_check=True)
                                    hgs.append(hg)
                                gwc = 4 * gw
                                t2 = bp.tile([P, 4 * KGRP], f32, tag="t2", bufs=3)
                                nc.scalar.activation(out=t2[:, 0:gwc], in_=ade[:, 0:gwc],
                                                     func=AF.Identity, scale=0.2)
                                t3 = bp.tile([P, 4 * KGRP], f32, tag="t3", bufs=3)
                                nc.vector.tensor_tensor(out=t3[:, 0:gwc], in0=ade[:, 0:gwc],
                                                        in1=t2[:, 0:gwc], op=ALU.max)
                                exf = bp.tile([P, 4 * KGRP], f32, tag="exf", bufs=3)
                                nc.scalar.activation(out=exf[:, 0:gwc], in_=t3[:, 0:gwc], func=AF.Exp)
                                exb = bp.tile([P, 4 * KGRP], bf16, tag="exb", bufs=3)
                                nc.vector.tensor_copy(out=exb[:, 0:gwc], in_=exf[:, 0:gwc])
                                for ci in range(gw):
                                    c = g0 + ci
                                    hg = hgs[ci]
                                    hgw = bp.tile([P, C], bf16, tag="hgw", bufs=4, name="hgw")
                                    nc.vector.tensor_tensor(
                                        out=hgw[:, 0:C].rearrange("p (h c) -> p h c", h=HEADS),
                                        in0=hg[:, 0:C].rearrange("p (h c) -> p h c", h=HEADS),
                                        in1=exb[:, ci * 4 : ci * 4 + 4, None].to_broadcast([P, HEADS, HID]),
                                        op=ALU.mult)
                                    nc.tensor.matmul(out=ps_out[:], lhsT=st4[:, ci, :], rhs=hgw[:],
                                                     start=(c == 0), stop=(c == nct - 1))
                                    nc.tensor.matmul(out=ps_den[:, 0:4], lhsT=st4[:, ci, :],
                                                     rhs=exb[:, ci * 4 : ci * 4 + 4],
                                                     start=(c == 0), stop=(c == nct - 1))
                            # -------- epilogue for node tile j --------
                            den = bp.tile([P, 4], f32, tag="den")
                            nc.vector.tensor_scalar(out=den[:], in0=ps_den[:, 0:4], scalar1=1e-30,
                                                    scalar2=None, op0=ALU.max)
                            rec = bp.tile([P, 4], f32, tag="rec")
                            nc.vector.reciprocal(out=rec[:], in_=den[:])
                            xl = bp.tile([P, C], f32, tag="xl", bufs=1)
                            for h in range(HEADS):
                                nc.vector.tensor_scalar(
                                    out=xl[:, h * HID : (h + 1) * HID],
                                    in0=ps_out[:, h * HID : (h + 1) * HID],
                                    scalar1=rec[:, h : h + 1], scalar2=None, op0=ALU.mult)
                            nc.vector.tensor_tensor(out=xl[:], in0=xl[:], in1=brep[:], op=ALU.add)
                            xr = bp.tile([P, C], f32r, tag="xr", bufs=1)
                            nc.scalar.activation(out=xr[:], in_=xl[:], func=AF.Relu)
                            xrb = bp.tile([P, C], bf16, tag="xrb", bufs=2)
                            nc.scalar.activation(out=xrb[:], in_=xl[:], func=AF.Relu)
                            nc.sync.dma_start(out=d_x[l][j * P : (j + 1) * P, :], in_=xrb[:])
                            tsbs = []
                            for kc in range(4):
                                tp = psum.tile([P, P], f32r, tag="psd", bufs=1, name="tp")
                                nc.tensor.transpose(out=tp[:], in_=xr[:, kc * P : (kc + 1) * P], identity=ident[:])
                                tsb = bp.tile([P, P], bf16, tag=f"tsb{kc}", name=f"tsb{kc}")
                                nc.vector.tensor_copy(out=tsb[:], in_=tp[:])
                                nc.sync.dma_start(
                                    out=d_xt[l][kc * P : (kc + 1) * P, j * P : (j + 1) * P], in_=tsb[:])
                                tsbs.append(tsb)
                            if l < L - 1:
                                psA = psum.tile([P, C], f32, tag="ade", bufs=2, name="psA")
                                ps8 = psum.tile([P, 8], f32, tag="pso", bufs=2, name="ps8")
                                for kc in range(4):
                                    nc.tensor.matmul(out=psA[:], lhsT=tsbs[kc][:], rhs=Wn[:, kc, :],
                                                     start=(kc == 0), stop=(kc == 3),
                                                     skip_group_check=True)
                                    nc.tensor.matmul(out=ps8[:], lhsT=tsbs[kc][:], rhs=Wadn[:, kc, :],
                                                     start=(kc == 0), stop=(kc == 3),
                                                     skip_group_check=True)
                                hsh = bp.tile([P, DH], bf16, tag="hsh")
                                nc.scalar.copy(out=hsh[:, 0:C], in_=psA[:])
                                nc.scalar.copy(out=hsh[:, C : C + 4], in_=ps8[:, 0:4])
                                nc.scalar.copy(out=adtab[l + 1][:, j * 4 : (j + 1) * 4], in_=ps8[:, 4:8])
                                nc.sync.dma_start(out=d_hsh[l + 1][j * P : (j + 1) * P, :], in_=hsh[:])
                            tc.tile_set_cur_wait(0.0)
                            if l < L - 1 and "ag" not in skip and j in (4, NTILES_OWN - 1):
                                half = 0 if j == 4 else 1
                                hn = (NTILES_OWN // 2) * P
                                nc.gpsimd.collective_compute(
                                    "AllGather", ALU.bypass, replica_groups=RG,
                                    ins=[d_hsh[l + 1][half * hn : half * hn + hn, :]],
                                    outs=[d_haug[l + 1][half * NCORES * hn : (half + 1) * NCORES * hn, :]])
                            if "lstm" not in skip:
                                if l >= 1 and j == 3:
                                    emit_lstm_block(0, l - 1, 2, wih=wihf, whh=whhf)
                                if l == L - 1:
                                    # this layer's own LSTM step, block by block,
                                    # as its d_xt columns land
                                    if j == 3:
                                        emit_lstm_block(0, 2, 0, wih=wihf, whh=whhf)
                                    elif j == 7:
                                        emit_lstm_block(0, 2, 1, wih=wihf, whh=whhf)
                                    elif j == 9:
                                        emit_lstm_block(0, 2, 2, wih=wihf, whh=whhf)
                                        for b in range(3):
                                            emit_lstm_block(1, 0, b, wih_dram=d_wih[1],
                                                            xtt_reuse=lstm_state[0]["xtt"])

            # ================= reverse LSTM (steps 1-2) =================
            if "lstm" not in skip:
                with tc.tile_pool(name="lstmwr", bufs=1) as wrp:
                    wihr = wrp.tile([P, 4, 4 * HL], bf16, tag="wihr", name="wihr")
                    nc.sync.dma_start(out=wihr[:], in_=d_wih[1].rearrange("(k p) g -> p k g", p=P))
                    whhr = wrp.tile([P, 6, 4 * HL], bf16, tag="whhr", name="whhr")
                    nc.sync.dma_start(out=whhr[:], in_=d_whh[1].rearrange("(k p) g -> p k g", p=P))
                    for step in (1, 2):
                        for b in range(3):
                            emit_lstm_block(1, step, b, wih=wihr, whh=whhr)

            # ================= JK attention + pooling =================
            with tc.tile_pool(name="jk", bufs=2) as jp:
                poolmat = jp.tile([P, NTILES_OWN, G], f32r, tag="pm")
                nc.sync.dma_start(out=poolmat[:], in_=d_poolmat[:, :, :])
                pool_ps = psum.tile([G, C], f32, tag="pso", bufs=2, name="pool_ps")
                for j in range(NTILES_OWN):
                    sc6 = jp.tile([P, 6], f32, tag="sc6")
                    nc.sync.dma_start(
                        out=sc6[:], in_=d_scores[:, j * P : (j + 1) * P].rearrange("s p -> p s"))
                    sc = jp.tile([P, 3], f32, tag="sc")
                    nc.vector.tensor_tensor(out=sc[:], in0=sc6[:, 0:3], in1=sc6[:, 3:6], op=ALU.add)
                    ex3 = jp.tile([P, 3], f32, tag="ex3")
                    nc.scalar.activation(out=ex3[:], in_=sc[:], func=AF.Exp)
                    s1 = jp.tile([P, 1], f32, tag="s1")
                    nc.vector.tensor_reduce(out=s1[:], in_=ex3[:], axis=mybir.AxisListType.X, op=ALU.add)
                    rec = jp.tile([P, 1], f32, tag="rec1")
                    nc.vector.reciprocal(out=rec[:], in_=s1[:])
                    alpha = jp.tile([P, 3], f32, tag="alpha")
                    nc.vector.tensor_scalar(out=alpha[:], in0=ex3[:], scalar1=rec[:, 0:1],
                                            scalar2=None, op0=ALU.mult)
                    acc = None
                    for t in range(3):
                        xlt = jp.tile([P, C], bf16, tag=f"xlt{t}", name=f"xlt{t}")
                        nc.sync.dma_start(out=xlt[:], in_=d_x[t][j * P : (j + 1) * P, :])
                        w = jp.tile([P, C], f32 if t < 2 else f32r, tag=f"w{t}", name=f"w{t}")
                        nc.vector.tensor_scalar(out=w[:], in0=xlt[:], scalar1=alpha[:, t : t + 1],
                                                scalar2=None, op0=ALU.mult)
                        if t == 0:
                            acc = w
                        elif t == 1:
                            nc.vector.tensor_tensor(out=acc[:], in0=acc[:], in1=w[:], op=ALU.add)
                        else:
                            xjk = jp.tile([P, C], f32r, tag="xjk")
                            nc.vector.tensor_tensor(out=xjk[:], in0=acc[:], in1=w[:], op=ALU.add)
                    nc.tensor.matmul(out=pool_ps[:], lhsT=poolmat[:, j, :], rhs=xjk[:],
                                     start=(j == 0), stop=(j == NTILES_OWN - 1))
                pool_sb = jp.tile([G, C], f32, tag="poolsb")
                nc.vector.tensor_copy(out=pool_sb[:], in_=pool_ps[:])
                nc.sync.dma_start(out=d_poolin[:, :], in_=pool_sb[:])
                nc.gpsimd.collective_compute(
                    "AllReduce", ALU.add, replica_groups=RG,
                    ins=[d_poolin.ap()], outs=[d_pooled.ap()])

            # ================= MLP =================
            with tc.tile_pool(name="mlp", bufs=1) as mp:
                fc1 = mp.tile([P, 4, C], f32r, tag="fc1")
                fc2 = mp.tile([P, 4, C], f32r, tag="fc2")
                for kc in range(4):
                    nc.sync.dma_start(out=fc1[:, kc, :], in_=d_fc1[kc * P : (kc + 1) * P, :])
                    nc.sync.dma_start(out=fc2[:, kc, :], in_=d_fc2[kc * P : (kc + 1) * P, :])
                fc3 = mp.tile([P, 4, OUT], f32r, tag="fc3")
                for kc in range(4):
                    nc.sync.dma_start(out=fc3[:, kc, :], in_=d_fc3[kc * P : (kc + 1) * P, :])
                fcb = mp.tile([P, 8], f32, tag="fcb")
                nc.sync.dma_start(out=fcb[:], in_=d_fcb[:, :])
                fc3b = mp.tile([OUT, 1], f32, tag="fc3b")
                nc.sync.dma_start(out=fc3b[:], in_=d_fc3b[:, :])
                plf = mp.tile([G, C], f32, tag="plf")
                nc.sync.dma_start(out=plf[:], in_=d_pooled[:, :])
                pl = mp.tile([G, C], f32r, tag="pl")
                nc.vector.tensor_copy(out=pl[:], in_=plf[:])
                gT = []
                for kc in range(4):
                    tp = psum.tile([P, G], f32r, tag="psd", bufs=1, name="mtp")
                    nc.tensor.transpose(out=tp[:, 0:G], in_=pl[0:G, kc * P : (kc + 1) * P],
                                        identity=ident[0:G, 0:G])
                    tsb = mp.tile([P, G], f32r, tag=f"gT{kc}", name=f"gT{kc}")
                    nc.vector.tensor_copy(out=tsb[:], in_=tp[:, 0:G])
                    gT.append(tsb)
                h1 = []
                for co in range(4):
                    ps = psum.tile([P, G], f32, tag="ade", bufs=2, name="mps1")
                    for kc in range(4):
                        nc.tensor.matmul(out=ps[:, 0:G], lhsT=fc1[:, kc, co * P : (co + 1) * P],
                                         rhs=gT[kc][:, 0:G], start=(kc == 0), stop=(kc == 3))
                    t = mp.tile([P, G], f32r, tag=f"h1{co}", name=f"h1{co}")
                    nc.scalar.activation(out=t[:], in_=ps[:, 0:G], func=AF.Relu,
                                         bias=fcb[:, co : co + 1])
                    h1.append(t)
                h2 = []
                for co in range(4):
                    ps = psum.tile([P, G], f32, tag="ade", bufs=2, name="mps2")
                    for kc in range(4):
                        nc.tensor.matmul(out=ps[:, 0:G], lhsT=fc2[:, kc, co * P : (co + 1) * P],
                                         rhs=h1[kc][:, 0:G], start=(kc == 0), stop=(kc == 3))
                    t = mp.tile([P, G], f32r, tag=f"h2{co}", name=f"h2{co}")
                    nc.scalar.activation(out=t[:], in_=ps[:, 0:G], func=AF.Relu,
                                         bias=fcb[:, 4 + co : 5 + co])
                    h2.append(t)
                ps = psum.tile([P, G], f32, tag="ade", bufs=2, name="mps3")
                for kc in range(4):
                    nc.tensor.matmul(out=ps[0:OUT, 0:G], lhsT=fc3[:, kc, :], rhs=h2[kc][:, 0:G],
                                     start=(kc == 0), stop=(kc == 3))
                osb = mp.tile([OUT, G], f32, tag="osb")
                nc.scalar.activation(out=osb[:], in_=ps[0:OUT, 0:G], func=AF.Identity,
                                     bias=fc3b[:, 0:1])
                nc.sync.dma_start(out=d_out[:, :], in_=osb[:])

    nc.compile()
    return nc


def build_in_maps(inputs, nct, srcidx, dlt, poolmat, stall, sall):
    inputs = {k: np.asarray(v) for k, v in inputs.items()}
    x = inputs["x"].astype(np.float32)
    xpad = np.zeros((NT, IN_C), np.float32)
    idx = np.arange(N)
    xpad[_gp(idx)] = x
    xT = np.ascontiguousarray(xpad.T)  # [128, NT]

    shared = {
        "xT": xT,
        "W0d": inputs["W0"].astype(np.float32),
        "fc1W": inputs["fc1_W"].astype(np.float32),
        "fc2W": inputs["fc2_W"].astype(np.float32),
        "fc3W": inputs["fc3_W"].astype(np.float32),
        "fc3b": inputs["fc3_b"].reshape(OUT, 1).astype(np.float32),
    }
    for l in range(L):
        shared[f"brep{l}"] = np.tile(inputs[f"b{l}"].reshape(1, C), (P, 1)).astype(np.float32)
    import ml_dtypes
    shared["W1d"] = inputs["W1"].astype(ml_dtypes.bfloat16)
    shared["W2d"] = inputs["W2"].astype(ml_dtypes.bfloat16)
    # attention coefficients folded into the weight matrices (host-side)
    W0 = inputs["W0"].astype(np.float32)
    asrc0 = inputs["asrc0"].astype(np.float32)
    adst0 = inputs["adst0"].astype(np.float32)
    shared["W0a"] = np.einsum("khc,hc->kh", W0.reshape(IN_C, HEADS, HID), asrc0).astype(np.float32)
    for l in (1, 2):
        Wl = inputs[f"W{l}"].astype(np.float32).reshape(C, HEADS, HID)
        wa = np.einsum("khc,hc->kh", Wl, inputs[f"asrc{l}"].astype(np.float32))
        wd = np.einsum("khc,hc->kh", Wl, inputs[f"adst{l}"].astype(np.float32))
        wad = np.concatenate([wa, wd], axis=1)  # [C, 8]
        shared[f"Wad{l}"] = np.ascontiguousarray(
            wad.reshape(4, P, 8).transpose(1, 0, 2)).astype(ml_dtypes.bfloat16)
    # host-precomputed destination attention table for layer 0 (per core below)
    h0ad = (x @ W0).reshape(N, HEADS, HID)
    a_d0 = np.einsum("nhc,hc->nh", h0ad, adst0).astype(np.float32)  # [N, 4]
    for i, d in enumerate("fr"):
        shared[f"WihT_{d}"] = np.ascontiguousarray(inputs[f"Wih_{d}"].T).astype(ml_dtypes.bfloat16)
        shared[f"WhhT_{d}"] = np.ascontiguousarray(inputs[f"Whh_{d}"].T).astype(ml_dtypes.bfloat16)
    bsum = np.zeros((P, 48), np.float32)
    for i, d in enumerate("fr"):
        bs = (inputs[f"bih_{d}"] + inputs[f"bhh_{d}"]).astype(np.float32)  # [3072]
        bsum[:, i * 24 : (i + 1) * 24] = bs.reshape(24, P).T
    shared["bsum"] = bsum
    attw = np.zeros((P, 12), np.float32)
    aw = inputs["att_w"].astype(np.float32)
    attw[:, 0:6] = aw[0:HL].reshape(6, P).T
    attw[:, 6:12] = aw[HL:].reshape(6, P).T
    shared["attw"] = attw.astype(ml_dtypes.bfloat16)
    fcb = np.zeros((P, 8), np.float32)
    fcb[:, 0:4] = inputs["fc1_b"].reshape(4, P).T
    fcb[:, 4:8] = inputs["fc2_b"].reshape(4, P).T
    shared["fcb"] = fcb

    in_maps = []
    for k in range(NCORES):
        m = dict(shared)
        own_ad = np.zeros((NPCP, 4), np.float32)
        own_ad[0:NPC] = a_d0[k * NPC : (k + 1) * NPC]
        m["adtab0"] = np.ascontiguousarray(
            own_ad.reshape(NTILES_OWN, P, 4).transpose(1, 0, 2).reshape(P, 4 * NTILES_OWN)
        ).astype(ml_dtypes.bfloat16)
        m["srcidx"] = srcidx[k]
        m["dlt"] = dlt[k]
        m["stall"] = stall[k]
        m["sall"] = sall[k]
        m["poolmat"] = poolmat[k]
        in_maps.append(m)
    return in_maps


def get_kernel(nct):
    if nct not in _CACHE:
        nswdge = int(os.environ.get("KERNEL_NSWDGE", "1"))
        _CACHE[nct] = build_nc(nct, nswdge=nswdge)
    return _CACHE[nct]


def kernel(**inputs):
    nct, srcidx, dlt, poolmat, stall, sall = build_tables(inputs["edge_index"], inputs["batch"])
    nc = get_kernel(nct)
    in_maps = build_in_maps(inputs, nct, srcidx, dlt, poolmat, stall, sall)
    from concourse.bass_utils import run_bass_kernel_spmd

    res = run_bass_kernel_spmd(nc, in_maps, core_ids=list(range(NCORES)))
    out_T = res.results[0]["out_T"]
    return np.ascontiguousarray(out_T.T.astype(np.float32))


# revision 47
# speedup vs baseline: 1.1365x; 1.1365x over previous
"""Trainium2 Bass kernel for GAT(3 layers, 4 heads) + JK-LSTM + global pool + MLP.

Sharding: nodes (and their incoming edges) are partitioned across 8 NeuronCores.
Layer-0 node transform is computed replicated (input x is replicated); layers 1-2
exchange activations via AllGather (bf16). Segment softmax + scatter-add
aggregation is done with one-hot matmuls over destination-sorted edge chunks.
The JK-LSTM (bf16 weights resident in SBUF), attention, pooling and MLP are
data-parallel over the node shard, with a final AllReduce for the graph pooling.
"""
import os
import sys

for _p in ("/opt/trn_rl_repo", "/root/.axon_site/_ro/trn_rl_repo"):
    if os.path.isdir(_p) and _p not in sys.path:
        sys.path.append(_p)

import numpy as np

import concourse.bass as bass
import concourse.bacc as bacc
import concourse.mybir as mybir
import concourse.tile as tile

P = 128
N, E, G = 10000, 160000, 64
IN_C, HID, HEADS, OUT = 128, 128, 4, 8
C, L, HL = 512, 3, 768
NCORES = 8
NPC = N // NCORES          # 1250 nodes per core
NPCP = 1280                # padded (10 tiles of 128)
NTILES_OWN = NPCP // P     # 10
NT = NPCP * NCORES         # 10240 padded total
NTILES_ALL = NT // P       # 80
DH = 520                   # haug row: h(512) | a_s(4) | pad(4)
SENT = 255.0               # sentinel dest-local for padding edges

f32 = mybir.dt.float32
bf16 = mybir.dt.bfloat16
f32r = mybir.dt.float32r
i32 = mybir.dt.int32
AF = mybir.ActivationFunctionType
ALU = mybir.AluOpType

_CACHE = {}


AGSPLIT = int(os.environ.get("KERNEL_AGSPLIT", "2"))


def _gp(n):
    """node id -> padded global slot (chunk-major so AllGather chunks are
    contiguous: [chunk][core][NPCP/chunks])"""
    n = np.asarray(n)
    k = n // NPC
    s = n % NPC
    cw = NPCP // AGSPLIT
    ch = s // cw
    return ch * (NCORES * cw) + k * cw + (s % cw)


def build_tables(edge_index, batch):
    """Host-side preprocessing: destination-sorted, per-(core,tile) chunked edge
    tables, pooling matrix."""
    ei = np.concatenate(
        [np.asarray(edge_index), np.tile(np.arange(N, dtype=np.int32), (2, 1))], axis=1
    )
    s_arr, d_arr = ei[0].astype(np.int64), ei[1].astype(np.int64)
    own = d_arr // NPC
    per_ct = {}
    for k in range(NCORES):
        m = own == k
        sk, dk = s_arr[m], d_arr[m]
        dloc = dk - NPC * k
        t_all = dloc // P
        for t in range(NTILES_OWN):
            tm = t_all == t
            per_ct[(k, t)] = (sk[tm], dloc[tm] - t * P)
    nct = max((len(v[0]) + P - 1) // P for v in per_ct.values())
    srcidx = np.zeros((NCORES, P, NTILES_OWN * nct), np.int32)
    dlt = np.full((NCORES, P, NTILES_OWN * nct), SENT, np.float32)
    for k in range(NCORES):
        for t in range(NTILES_OWN):
            sk, dloc = per_ct[(k, t)]
            ne = len(sk)
            col0 = t * nct
            for c in range((ne + P - 1) // P):
                lo, hi = c * P, min((c + 1) * P, ne)
                srcidx[k, 0 : hi - lo, col0 + c] = _gp(sk[lo:hi])
                dlt[k, 0 : hi - lo, col0 + c] = dloc[lo:hi]
    # precomputed one-hot scatter masks, both orientations
    ncols = NTILES_OWN * nct
    import ml_dtypes
    ar = np.arange(P, dtype=np.float32)
    stall = np.zeros((NCORES, ncols, P, P), ml_dtypes.bfloat16)
    sall = np.zeros((NCORES, ncols, P, P), ml_dtypes.bfloat16)
    for k in range(NCORES):
        stk = (dlt[k].T[:, :, None] == ar[None, None, :]).astype(ml_dtypes.bfloat16)  # [ncols, e, n]
        stall[k] = stk
        sall[k] = stk.transpose(0, 2, 1)
    batch = np.asarray(batch)
    cnt = np.maximum(np.bincount(batch, minlength=G), 1).astype(np.float32)
    poolmat = np.zeros((NCORES, P, NTILES_OWN, G), np.float32)
    inv = 1.0 / cnt
    for n in range(N):
        k, sl = n // NPC, n % NPC
        poolmat[k, sl % P, sl // P, batch[n]] = inv[batch[n]]
    return nct, srcidx, dlt, poolmat, stall, sall


def build_nc(nct, skip=frozenset(), nswdge=1):
    NCH = NTILES_OWN * nct
    nc = bacc.Bacc("TRN2", target_bir_lowering=False, debug=False, num_devices=NCORES,
                   num_swdge_queues=nswdge)

    # ---------------- kernel I/O ----------------
    d_xT = nc.dram_tensor("xT", [P, NT], f32r, kind="ExternalInput")
    d_W0a = nc.dram_tensor("W0a", [P, 4], f32r, kind="ExternalInput")
    d_adtab0 = nc.dram_tensor("adtab0", [P, 4 * NTILES_OWN], bf16, kind="ExternalInput")
    d_Wad = [None,
             nc.dram_tensor("Wad1", [P, 4, 8], bf16, kind="ExternalInput"),
             nc.dram_tensor("Wad2", [P, 4, 8], bf16, kind="ExternalInput")]
    d_W = [
        nc.dram_tensor("W0d", [P, C], f32r, kind="ExternalInput"),
        nc.dram_tensor("W1d", [C, C], bf16, kind="ExternalInput"),
        nc.dram_tensor("W2d", [C, C], bf16, kind="ExternalInput"),
    ]
    d_brep = [nc.dram_tensor(f"brep{l}", [P, C], f32, kind="ExternalInput") for l in range(L)]
    d_srcidx = nc.dram_tensor("srcidx", [P, NCH], i32, kind="ExternalInput")
    d_dlt = nc.dram_tensor("dlt", [P, NCH], f32, kind="ExternalInput")
    d_stall = nc.dram_tensor("stall", [NCH, P, P], bf16, kind="ExternalInput")
    d_sall = nc.dram_tensor("sall", [NCH, P, P], bf16, kind="ExternalInput")
    d_wih = [nc.dram_tensor(f"WihT_{d}", [C, 4 * HL], bf16, kind="ExternalInput") for d in "fr"]
    d_whh = [nc.dram_tensor(f"WhhT_{d}", [HL, 4 * HL], bf16, kind="ExternalInput") for d in "fr"]
    d_bsum = nc.dram_tensor("bsum", [P, 48], f32, kind="ExternalInput")
    d_attw = nc.dram_tensor("attw", [P, 12], bf16, kind="ExternalInput")
    d_poolmat = nc.dram_tensor("poolmat", [P, NTILES_OWN, G], f32r, kind="ExternalInput")
    d_fc1 = nc.dram_tensor("fc1W", [C, C], f32r, kind="ExternalInput")
    d_fc2 = nc.dram_tensor("fc2W", [C, C], f32r, kind="ExternalInput")
    d_fc3 = nc.dram_tensor("fc3W", [C, OUT], f32r, kind="ExternalInput")
    d_fcb = nc.dram_tensor("fcb", [P, 8], f32, kind="ExternalInput")  # fc1_b | fc2_b
    d_fc3b = nc.dram_tensor("fc3b", [OUT, 1], f32, kind="ExternalInput")
    d_out = nc.dram_tensor("out_T", [OUT, G], f32, kind="ExternalOutput")

    # ---------------- internal DRAM ----------------
    d_haug = [
        nc.dram_tensor("haug0", [NT, DH], bf16),
        nc.dram_tensor("haug1", [NT, DH], bf16, addr_space="Shared"),
        nc.dram_tensor("haug2", [NT, DH], bf16, addr_space="Shared"),
    ]
    d_hsh = [None, nc.dram_tensor("hsh1", [NPCP, DH], bf16), nc.dram_tensor("hsh2", [NPCP, DH], bf16)]
    d_x = [nc.dram_tensor(f"x_l{l}", [NPCP, C], bf16) for l in range(L)]
    d_xt = [nc.dram_tensor(f"xt_l{l}", [C, NPCP], bf16) for l in range(L)]
    d_scores = nc.dram_tensor("scoresd", [6, NPCP], f32)
    d_poolin = nc.dram_tensor("poolin", [G, C], f32)
    d_pooled = nc.dram_tensor("pooled", [G, C], f32, addr_space="Shared")

    RG = [list(range(NCORES))]
    BLKS = [(0, 512), (512, 512), (1024, 256)]  # node blocks of NPCP
    # j-tiles after which an AllGather chunk is issued; the _gp slot layout
    # must use the matching number of chunks
    AG_AT = {1: (NTILES_OWN - 1,), 2: (4, NTILES_OWN - 1),
             5: (1, 3, 5, 7, 9)}[AGSPLIT]

    with tile.TileContext(nc) as tc, \
         nc.allow_low_precision(reason="bf16 activations within tolerance"):
        with tc.tile_pool(name="const", bufs=1) as const, \
             tc.tile_pool(name="psum", bufs=1, space="PSUM") as psum, \
             tc.tile_pool(name="lstmp", bufs=1) as lstmp, \
             tc.tile_pool(name="lstmx", bufs=2) as lstmx:
            ident_f = const.tile([P, P], f32)
            from concourse.masks import make_identity

            make_identity(nc, ident_f[:])
            ident = const.tile([P, P], f32r)
            nc.vector.tensor_copy(out=ident[:], in_=ident_f[:])
            identb = const.tile([P, P], bf16)
            nc.vector.tensor_copy(out=identb[:], in_=ident_f[:])
            srcidx = const.tile([P, NCH], i32)
            nc.sync.dma_start(out=srcidx[:], in_=d_srcidx[:, :])
            adtab = [const.tile([P, 4 * NTILES_OWN], bf16, tag=f"adtab{_l}", name=f"adtab{_l}") for _l in range(L)]
            bsum = const.tile([P, 48], f32)
            nc.sync.dma_start(out=bsum[:], in_=d_bsum[:, :])
            attw = const.tile([P, 12], bf16)
            nc.sync.dma_start(out=attw[:], in_=d_attw[:, :])

            # ---------- LSTM state (persistent across sections) ----------
            lstm_state = {}

            def emit_lstm_block(dire, step, b, wih=None, whh=None, wih_dram=None,
                                xtt_reuse=None):
                """Emit one node-block of one LSTM step. Blocks are pointwise-
                independent so they can interleave with GAT tile processing."""
                t = step if dire == 0 else 2 - step
                b0, bw = BLKS[b]
                st_ = lstm_state.setdefault(dire, {"h": [None] * 6})
                if step == 0 and b == 0:
                    st_["c"] = [lstmp.tile([P, NPCP], bf16, tag=f"c{dire}{j}", name=f"c{dire}{j}")
                                for j in range(6)]
                cst = st_["c"]
                if b == 0:
                    st_["h"] = st_.get("hn", [None] * 6)
                    st_["hn"] = [lstmp.tile([P, NPCP], bf16, tag=f"h{j}", bufs=2, name=f"h{j}")
                                 for j in range(6)]
                    st_["sc"] = lstmp.tile([1, NPCP], f32, tag=f"sc{dire}", name=f"sc{dire}")
                    nc.vector.memset(st_["sc"][:], 0.0)
                    if xtt_reuse is not None:
                        st_["xtt"] = xtt_reuse
                    else:
                        st_["xtt"] = lstmx.tile([P, 4, NPCP], bf16, tag="xtt", bufs=2, name="xtt")
                h_prev, hn, sc_acc, xtt = st_["h"], st_["hn"], st_["sc"], st_["xtt"]
                if xtt_reuse is None:
                    nc.sync.dma_start(
                        out=xtt[:, :, b0 : b0 + bw],
                        in_=d_xt[t][:, b0 : b0 + bw].rearrange("(k p) n -> p k n", p=P))
                gates = (0, 2, 3) if step == 0 else (0, 1, 2, 3)
                for j in range(6):
                    gas = {}
                    for gate in gates:
                        gt_row = gate * 6 + j
                        if wih_dram is not None:
                            wt = lstmx.tile([P, 4, P], bf16, tag="wihs", bufs=6, name="wihs")
                            nc.sync.dma_start(
                                out=wt[:],
                                in_=wih_dram[:, gt_row * P : (gt_row + 1) * P]
                                .rearrange("(k p) g -> p k g", p=P))
                            wslc = lambda kc: wt[:, kc, :]
                        else:
                            wslc = lambda kc: wih[:, kc, gt_row * P : (gt_row + 1) * P]
                        ga = lstmp.tile([P, 512], bf16, tag=f"ga{gate}", bufs=2, name=f"ga{gate}")
                        gps = psum.tile([P, 512], f32, tag="gps", bufs=2, name="gps")
                        for kc in range(4):
                            nc.tensor.matmul(
                                out=gps[:, 0:bw], lhsT=wslc(kc),
                                rhs=xtt[:, kc, b0 : b0 + bw],
                                start=(kc == 0), stop=(kc == 3 and step == 0))
                        if step > 0:
                            for kc in range(6):
                                nc.tensor.matmul(
                                    out=gps[:, 0:bw],
                                    lhsT=whh[:, kc, gt_row * P : (gt_row + 1) * P],
                                    rhs=h_prev[kc][:, b0 : b0 + bw],
                                    start=False, stop=(kc == 5))
                        nc.scalar.activation(
                            out=ga[:, 0:bw], in_=gps[:, 0:bw],
                            func=(AF.Tanh if gate == 2 else AF.Sigmoid),
                            bias=bsum[:, dire * 24 + gt_row : dire * 24 + gt_row + 1])
                        gas[gate] = ga
                    csl = cst[j][:, b0 : b0 + bw]
                    if step == 0:
                        nc.vector.tensor_tensor(out=csl, in0=gas[0][:, 0:bw],
                                                in1=gas[2][:, 0:bw], op=ALU.mult)
                    else:
                        nc.vector.tensor_tensor(out=gas[0][:, 0:bw], in0=gas[0][:, 0:bw],
                                                in1=gas[2][:, 0:bw], op=ALU.mult)
                        nc.vector.tensor_tensor(out=csl, in0=csl, in1=gas[1][:, 0:bw], op=ALU.mult)
                        nc.vector.tensor_tensor(out=csl, in0=csl, in1=gas[0][:, 0:bw], op=ALU.add)
                    tnh = gas[2]
                    nc.scalar.activation(out=tnh[:, 0:bw], in_=csl, func=AF.Tanh)
                    nc.vector.tensor_tensor(out=hn[j][:, b0 : b0 + bw], in0=tnh[:, 0:bw],
                                            in1=gas[3][:, 0:bw], op=ALU.mult)
                    scp = psum.tile([1, 512], f32, tag="scp", bufs=1, name="scp")
                    nc.tensor.matmul(
                        out=scp[:, 0:bw],
                        lhsT=attw[:, dire * 6 + j : dire * 6 + j + 1],
                        rhs=hn[j][:, b0 : b0 + bw], start=True, stop=True)
                    nc.vector.tensor_tensor(
                        out=sc_acc[0:1, b0 : b0 + bw], in0=sc_acc[0:1, b0 : b0 + bw],
                        in1=scp[:, 0:bw], op=ALU.add)
                nc.sync.dma_start(out=d_scores[dire * 3 + t, b0 : b0 + bw][None, :],
                                  in_=sc_acc[0:1, b0 : b0 + bw])

            # ============ forward-direction LSTM weights (resident) ============
            with tc.tile_pool(name="lstmwf", bufs=1) as wfp:
                wihf = wfp.tile([P, 4, 4 * HL], bf16, tag="wihf", name="wihf")
                nc.sync.dma_start(out=wihf[:], in_=d_wih[0].rearrange("(k p) g -> p k g", p=P))
                whhf = wfp.tile([P, 6, 4 * HL], bf16, tag="whhf", name="whhf")
                nc.sync.dma_start(out=whhf[:], in_=d_whh[0].rearrange("(k p) g -> p k g", p=P))

                # ================= stage A0 =================
                with tc.tile_pool(name="a0", bufs=2) as a0p:
                    W0t = a0p.tile([P, C], f32r, tag="w0", bufs=1)
                    nc.sync.dma_start(out=W0t[:], in_=d_W[0][:, :])
                    W0a = a0p.tile([P, 4], f32r, tag="w0a", bufs=1)
                    nc.sync.dma_start(out=W0a[:], in_=d_W0a[:, :])
                    nc.sync.dma_start(out=adtab[0][:], in_=d_adtab0[:, :])
                    for nt in range(NTILES_ALL):
                        xt_t = a0p.tile([P, P], f32r, tag="xt", bufs=4)
                        nc.sync.dma_start(out=xt_t[:], in_=d_xT[:, nt * P : (nt + 1) * P])
                        ps = psum.tile([P, C], f32, tag="ade", bufs=2, name="psa0")
                        nc.tensor.matmul(out=ps[:], lhsT=xt_t[:], rhs=W0t[:], start=True, stop=True)
                        ps8 = psum.tile([P, 8], f32, tag="pso", bufs=2, name="psa0a")
                        nc.tensor.matmul(out=ps8[:, 0:4], lhsT=xt_t[:], rhs=W0a[:], start=True, stop=True)
                        ht = a0p.tile([P, DH], bf16, tag="ht", bufs=4)
                        nc.scalar.copy(out=ht[:, 0:C], in_=ps[:])
                        nc.scalar.copy(out=ht[:, C : C + 4], in_=ps8[:, 0:4])
                        nc.sync.dma_start(out=d_haug[0][nt * P : (nt + 1) * P, :], in_=ht[:])

                # ======== GAT layers (+ interleaved fwd-LSTM steps) ========
                KGRP = 4
                for l in range(L):
                    with tc.tile_pool(name=f"b{l}", bufs=2) as bp, \
                         tc.tile_pool(name=f"b{l}g", bufs=10) as bg:
                        brep = bp.tile([P, C], f32, tag="brep", bufs=1)
                        nc.sync.dma_start(out=brep[:], in_=d_brep[l][:, :])
                        if l < L - 1:
                            Wn = bp.tile([P, 4, C], bf16, tag="wn", bufs=1)
                            for kc in range(4):
                                nc.sync.dma_start(out=Wn[:, kc, :], in_=d_W[l + 1][kc * P : (kc + 1) * P, :])
                            Wadn = bp.tile([P, 4, 8], bf16, tag="wadn", bufs=1)
                            nc.sync.dma_start(out=Wadn[:], in_=d_Wad[l + 1][:, :, :])
                        # hide the inbound AllGather: run the previous layer's
                        # LSTM step (blocks 0-1) before this layer's gathers
                        if l >= 1 and "lstm" not in skip:
                            emit_lstm_block(0, l - 1, 0, wih=wihf, whh=whhf)
                            emit_lstm_block(0, l - 1, 1, wih=wihf, whh=whhf)
                        for j in range(NTILES_OWN):
                            ps_out = psum.tile([P, C], f32, tag="pso", bufs=2, name="ps_out")
                            ps_den = psum.tile([P, C], f32, tag="psd", bufs=1, name="ps_den")
                            for g0 in range(0, nct, KGRP):
                                gw = min(KGRP, nct - g0)
                                colg = j * nct + g0
                                hgs = []
                                st4 = bp.tile([P, KGRP, P], bf16, tag="st4", bufs=3, name="st4")
                                nc.sync.dma_start(
                                    out=st4[:, 0:gw, :],
                                    in_=d_stall[colg : colg + gw].rearrange("c e n -> e c n"))
                                s4 = bp.tile([P, KGRP, P], bf16, tag="s4", bufs=3, name="s4")
                                nc.sync.dma_start(
                                    out=s4[:, 0:gw, :],
                                    in_=d_sall[colg : colg + gw].rearrange("c n e -> n c e"))
                                ade = psum.tile([P, C], f32, tag="ade", bufs=2, name="ade")
                                for ci in range(gw):
                                    col = colg + ci
                                    hg = bg.tile([P, DH], bf16, tag="hg", name="hg")
                                    nc.gpsimd.indirect_dma_start(
                                        out=hg[:], out_offset=None,
                                        in_=d_haug[0 if "ag" in skip else l][:, :],
                                        in_offset=bass.IndirectOffsetOnAxis(ap=srcidx[:, col : col + 1], axis=0))
                                    nc.tensor.matmul(out=ade[:, ci * 4 : ci * 4 + 4], lhsT=s4[:, ci, :],
                                                     rhs=adtab[l][:, j * 4 : (j + 1) * 4],
                                                     start=True, stop=False, skip_group_check=True)
                                    nc.tensor.matmul(out=ade[:, ci * 4 : ci * 4 + 4], lhsT=identb[:, :],
                                                     rhs=hg[:, C : C + 4],
                                                     start=False, stop=True, skip_group_check=True)
                                    hgs.append(hg)
                                gwc = 4 * gw
                                t2 = bp.tile([P, 4 * KGRP], f32, tag="t2", bufs=3)
                                nc.scalar.activation(out=t2[:, 0:gwc], in_=ade[:, 0:gwc],
                                                     func=AF.Identity, scale=0.2)
                                t3 = bp.tile([P, 4 * KGRP], f32, tag="t3", bufs=3)
                                nc.vector.tensor_tensor(out=t3[:, 0:gwc], in0=ade[:, 0:gwc],
                                                        in1=t2[:, 0:gwc], op=ALU.max)
                                exf = bp.tile([P, 4 * KGRP], f32, tag="exf", bufs=3)
                                nc.scalar.activation(out=exf[:, 0:gwc], in_=t3[:, 0:gwc], func=AF.Exp)
                                exb = bp.tile([P, 4 * KGRP], bf16, tag="exb", bufs=3)
                                nc.vector.tensor_copy(out=exb[:, 0:gwc], in_=exf[:, 0:gwc])
                                for ci in range(gw):
                                    c = g0 + ci
                                    hg = hgs[ci]
                                    hgw = bp.tile([P, C], bf16, tag="hgw", bufs=4, name="hgw")
                                    nc.vector.tensor_tensor(
                                        out=hgw[:, 0:C].rearrange("p (h c) -> p h c", h=HEADS),
                                        in0=hg[:, 0:C].rearrange("p (h c) -> p h c", h=HEADS),
                                        in1=exb[:, ci * 4 : ci * 4 + 4, None].to_broadcast([P, HEADS, HID]),
                                        op=ALU.mult)
                                    nc.tensor.matmul(out=ps_out[:], lhsT=st4[:, ci, :], rhs=hgw[:],
                                                     start=(c == 0), stop=(c == nct - 1))
                                    nc.tensor.matmul(out=ps_den[:, 0:4], lhsT=st4[:, ci, :],
                                                     rhs=exb[:, ci * 4 : ci * 4 + 4],
                                                     start=(c == 0), stop=(c == nct - 1))
                            # -------- epilogue for node tile j --------
                            den = bp.tile([P, 4], f32, tag="den")
                            nc.vector.tensor_scalar(out=den[:], in0=ps_den[:, 0:4], scalar1=1e-30,
                                                    scalar2=None, op0=ALU.max)
                            rec = bp.tile([P, 4], f32, tag="rec")
                            nc.vector.reciprocal(out=rec[:], in_=den[:])
                            xl = bp.tile([P, C], f32, tag="xl", bufs=1)
                            for h in range(HEADS):
                                nc.vector.tensor_scalar(
                                    out=xl[:, h * HID : (h + 1) * HID],
                                    in0=ps_out[:, h * HID : (h + 1) * HID],
                                    scalar1=rec[:, h : h + 1], scalar2=None, op0=ALU.mult)
                            nc.vector.tensor_tensor(out=xl[:], in0=xl[:], in1=brep[:], op=ALU.add)
                            xr = bp.tile([P, C], f32r, tag="xr", bufs=1)
                            nc.scalar.activation(out=xr[:], in_=xl[:], func=AF.Relu)
                            xrb = bp.tile([P, C], bf16, tag="xrb", bufs=2)
                            nc.scalar.activation(out=xrb[:], in_=xl[:], func=AF.Relu)
                            nc.sync.dma_start(out=d_x[l][j * P : (j + 1) * P, :], in_=xrb[:])
                            tsbs = []
                            for kc in range(4):
                                tp = psum.tile([P, P], f32r, tag="psd", bufs=1, name="tp")
                                nc.tensor.transpose(out=tp[:], in_=xr[:, kc * P : (kc + 1) * P], identity=ident[:])
                                tsb = bp.tile([P, P], bf16, tag=f"tsb{kc}", name=f"tsb{kc}")
                                nc.vector.tensor_copy(out=tsb[:], in_=tp[:])
                                nc.sync.dma_start(
                                    out=d_xt[l][kc * P : (kc + 1) * P, j * P : (j + 1) * P], in_=tsb[:])
                                tsbs.append(tsb)
                            if l < L - 1:
                                psA = psum.tile([P, C], f32, tag="ade", bufs=2, name="psA")
                                ps8 = psum.tile([P, 8], f32, tag="pso", bufs=2, name="ps8")
                                for kc in range(4):
                                    nc.tensor.matmul(out=psA[:], lhsT=tsbs[kc][:], rhs=Wn[:, kc, :],
                                                     start=(kc == 0), stop=(kc == 3),
                                                     skip_group_check=True)
                                    nc.tensor.matmul(out=ps8[:], lhsT=tsbs[kc][:], rhs=Wadn[:, kc, :],
                                                     start=(kc == 0), stop=(kc == 3),
                                                     skip_group_check=True)
                                hsh = bp.tile([P, DH], bf16, tag="hsh")
                                nc.scalar.copy(out=hsh[:, 0:C], in_=psA[:])
                                nc.scalar.copy(out=hsh[:, C : C + 4], in_=ps8[:, 0:4])
                                nc.scalar.copy(out=adtab[l + 1][:, j * 4 : (j + 1) * 4], in_=ps8[:, 4:8])
                                nc.sync.dma_start(out=d_hsh[l + 1][j * P : (j + 1) * P, :], in_=hsh[:])
                            if l < L - 1 and "ag" not in skip and j in AG_AT:
                                half = AG_AT.index(j)
                                hn = NPCP // len(AG_AT)
                                nc.gpsimd.collective_compute(
                                    "AllGather", ALU.bypass, replica_groups=RG,
                                    ins=[d_hsh[l + 1][half * hn : half * hn + hn, :]],
                                    outs=[d_haug[l + 1][half * NCORES * hn : (half + 1) * NCORES * hn, :]])
                            if "lstm" not in skip:
                                if l >= 1 and j == 3:
                                    emit_lstm_block(0, l - 1, 2, wih=wihf, whh=whhf)
                                if l == L - 1:
                                    # this layer's own LSTM step, block by block,
                                    # as its d_xt columns land
                                    if j == 3:
                                        emit_lstm_block(0, 2, 0, wih=wihf, whh=whhf)
                                    elif j == 7:
                                        emit_lstm_block(0, 2, 1, wih=wihf, whh=whhf)
                                    elif j == 9:
                                        emit_lstm_block(0, 2, 2, wih=wihf, whh=whhf)
                                        for b in range(3):
                                            emit_lstm_block(1, 0, b, wih_dram=d_wih[1],
                                                            xtt_reuse=lstm_state[0]["xtt"])

            # ================= reverse LSTM (steps 1-2) =================
            if "lstm" not in skip:
                with tc.tile_pool(name="lstmwr", bufs=1) as wrp:
                    wihr = wrp.tile([P, 4, 4 * HL], bf16, tag="wihr", name="wihr")
                    nc.sync.dma_start(out=wihr[:], in_=d_wih[1].rearrange("(k p) g -> p k g", p=P))
                    whhr = wrp.tile([P, 6, 4 * HL], bf16, tag="whhr", name="whhr")
                    nc.sync.dma_start(out=whhr[:], in_=d_whh[1].rearrange("(k p) g -> p k g", p=P))
                    for step in (1, 2):
                        for b in range(3):
                            emit_lstm_block(1, step, b, wih=wihr, whh=whhr)

            # ================= JK attention + pooling =================
            with tc.tile_pool(name="jk", bufs=2) as jp:
                poolmat = jp.tile([P, NTILES_OWN, G], f32r, tag="pm")
                nc.sync.dma_start(out=poolmat[:], in_=d_poolmat[:, :, :])
                pool_ps = psum.tile([G, C], f32, tag="pso", bufs=2, name="pool_ps")
                for j in range(NTILES_OWN):
                    sc6 = jp.tile([P, 6], f32, tag="sc6")
                    nc.sync.dma_start(
                        out=sc6[:], in_=d_scores[:, j * P : (j + 1) * P].rearrange("s p -> p s"))
                    sc = jp.tile([P, 3], f32, tag="sc")
                    nc.vector.tensor_tensor(out=sc[:], in0=sc6[:, 0:3], in1=sc6[:, 3:6], op=ALU.add)
                    ex3 = jp.tile([P, 3], f32, tag="ex3")
                    nc.scalar.activation(out=ex3[:], in_=sc[:], func=AF.Exp)
                    s1 = jp.tile([P, 1], f32, tag="s1")
                    nc.vector.tensor_reduce(out=s1[:], in_=ex3[:], axis=mybir.AxisListType.X, op=ALU.add)
                    rec = jp.tile([P, 1], f32, tag="rec1")
                    nc.vector.reciprocal(out=rec[:], in_=s1[:])
                    alpha = jp.tile([P, 3], f32, tag="alpha")
                    nc.vector.tensor_scalar(out=alpha[:], in0=ex3[:], scalar1=rec[:, 0:1],
                                            scalar2=None, op0=ALU.mult)
                    acc = None
                    for t in range(3):
                        xlt = jp.tile([P, C], bf16, tag=f"xlt{t}", name=f"xlt{t}")
                        nc.sync.dma_start(out=xlt[:], in_=d_x[t][j * P : (j + 1) * P, :])
                        w = jp.tile([P, C], f32 if t < 2 else f32r, tag=f"w{t}", name=f"w{t}")
                        nc.vector.tensor_scalar(out=w[:], in0=xlt[:], scalar1=alpha[:, t : t + 1],
                                                scalar2=None, op0=ALU.mult)
                        if t == 0:
                            acc = w
                        elif t == 1:
                            nc.vector.tensor_tensor(out=acc[:], in0=acc[:], in1=w[:], op=ALU.add)
                        else:
                            xjk = jp.tile([P, C], f32r, tag="xjk")
                            nc.vector.tensor_tensor(out=xjk[:], in0=acc[:], in1=w[:], op=ALU.add)
                    nc.tensor.matmul(out=pool_ps[:], lhsT=poolmat[:, j, :], rhs=xjk[:],
                                     start=(j == 0), stop=(j == NTILES_OWN - 1))
                pool_sb = jp.tile([G, C], f32, tag="poolsb")
                nc.vector.tensor_copy(out=pool_sb[:], in_=pool_ps[:])
                nc.sync.dma_start(out=d_poolin[:, :], in_=pool_sb[:])
                nc.gpsimd.collective_compute(
                    "AllReduce", ALU.add, replica_groups=RG,
                    ins=[d_poolin.ap()], outs=[d_pooled.ap()])

            # ================= MLP =================
            with tc.tile_pool(name="mlp", bufs=1) as mp:
                fc1 = mp.tile([P, 4, C], f32r, tag="fc1")
                fc2 = mp.tile([P, 4, C], f32r, tag="fc2")
                for kc in range(4):
                    nc.sync.dma_start(out=fc1[:, kc, :], in_=d_fc1[kc * P : (kc + 1) * P, :])
                    nc.sync.dma_start(out=fc2[:, kc, :], in_=d_fc2[kc * P : (kc + 1) * P, :])
                fc3 = mp.tile([P, 4, OUT], f32r, tag="fc3")
                for kc in range(4):
                    nc.sync.dma_start(out=fc3[:, kc, :], in_=d_fc3[kc * P : (kc + 1) * P, :])
                fcb = mp.tile([P, 8], f32, tag="fcb")
                nc.sync.dma_start(out=fcb[:], in_=d_fcb[:, :])
                fc3b = mp.tile([OUT, 1], f32, tag="fc3b")
                nc.sync.dma_start(out=fc3b[:], in_=d_fc3b[:, :])
                plf = mp.tile([G, C], f32, tag="plf")
                nc.sync.dma_start(out=plf[:], in_=d_pooled[:, :])
                pl = mp.tile([G, C], f32r, tag="pl")
                nc.vector.tensor_copy(out=pl[:], in_=plf[:])
                gT = []
                for kc in range(4):
                    tp = psum.tile([P, G], f32r, tag="psd", bufs=1, name="mtp")
                    nc.tensor.transpose(out=tp[:, 0:G], in_=pl[0:G, kc * P : (kc + 1) * P],
                                        identity=ident[0:G, 0:G])
                    tsb = mp.tile([P, G], f32r, tag=f"gT{kc}", name=f"gT{kc}")
                    nc.vector.tensor_copy(out=tsb[:], in_=tp[:, 0:G])
                    gT.append(tsb)
                h1 = []
                for co in range(4):
                    ps = psum.tile([P, G], f32, tag="ade", bufs=2, name="mps1")
                    for kc in range(4):
                        nc.tensor.matmul(out=ps[:, 0:G], lhsT=fc1[:, kc, co * P : (co + 1) * P],
                                         rhs=gT[kc][:, 0:G], start=(kc == 0), stop=(kc == 3))
                    t = mp.tile([P, G], f32r, tag=f"h1{co}", name=f"h1{co}")
                    nc.scalar.activation(out=t[:], in_=ps[:, 0:G], func=AF.Relu,
                                         bias=fcb[:, co : co + 1])
                    h1.append(t)
                h2 = []
                for co in range(4):
                    ps = psum.tile([P, G], f32, tag="ade", bufs=2, name="mps2")
                    for kc in range(4):
                        nc.tensor.matmul(out=ps[:, 0:G], lhsT=fc2[:, kc, co * P : (co + 1) * P],
                                         rhs=h1[kc][:, 0:G], start=(kc == 0), stop=(kc == 3))
                    t = mp.tile([P, G], f32r, tag=f"h2{co}", name=f"h2{co}")
                    nc.scalar.activation(out=t[:], in_=ps[:, 0:G], func=AF.Relu,
                                         bias=fcb[:, 4 + co : 5 + co])
                    h2.append(t)
                ps = psum.tile([P, G], f32, tag="ade", bufs=2, name="mps3")
                for kc in range(4):
                    nc.tensor.matmul(out=ps[0:OUT, 0:G], lhsT=fc3[:, kc, :], rhs=h2[kc][:, 0:G],
                                     start=(kc == 0), stop=(kc == 3))
                osb = mp.tile([OUT, G], f32, tag="osb")
                nc.scalar.activation(out=osb[:], in_=ps[0:OUT, 0:G], func=AF.Identity,
                                     bias=fc3b[:, 0:1])
                nc.sync.dma_start(out=d_out[:, :], in_=osb[:])

    nc.compile()
    return nc


def build_in_maps(inputs, nct, srcidx, dlt, poolmat, stall, sall):
    inputs = {k: np.asarray(v) for k, v in inputs.items()}
    x = inputs["x"].astype(np.float32)
    xpad = np.zeros((NT, IN_C), np.float32)
    idx = np.arange(N)
    xpad[_gp(idx)] = x
    xT = np.ascontiguousarray(xpad.T)  # [128, NT]

    shared = {
        "xT": xT,
        "W0d": inputs["W0"].astype(np.float32),
        "fc1W": inputs["fc1_W"].astype(np.float32),
        "fc2W": inputs["fc2_W"].astype(np.float32),
        "fc3W": inputs["fc3_W"].astype(np.float32),
        "fc3b": inputs["fc3_b"].reshape(OUT, 1).astype(np.float32),
    }
    for l in range(L):
        shared[f"brep{l}"] = np.tile(inputs[f"b{l}"].reshape(1, C), (P, 1)).astype(np.float32)
    import ml_dtypes
    shared["W1d"] = inputs["W1"].astype(ml_dtypes.bfloat16)
    shared["W2d"] = inputs["W2"].astype(ml_dtypes.bfloat16)
    # attention coefficients folded into the weight matrices (host-side)
    W0 = inputs["W0"].astype(np.float32)
    asrc0 = inputs["asrc0"].astype(np.float32)
    adst0 = inputs["adst0"].astype(np.float32)
    shared["W0a"] = np.einsum("khc,hc->kh", W0.reshape(IN_C, HEADS, HID), asrc0).astype(np.float32)
    for l in (1, 2):
        Wl = inputs[f"W{l}"].astype(np.float32).reshape(C, HEADS, HID)
        wa = np.einsum("khc,hc->kh", Wl, inputs[f"asrc{l}"].astype(np.float32))
        wd = np.einsum("khc,hc->kh", Wl, inputs[f"adst{l}"].astype(np.float32))
        wad = np.concatenate([wa, wd], axis=1)  # [C, 8]
        shared[f"Wad{l}"] = np.ascontiguousarray(
            wad.reshape(4, P, 8).transpose(1, 0, 2)).astype(ml_dtypes.bfloat16)
    # host-precomputed destination attention table for layer 0 (per core below)
    h0ad = (x @ W0).reshape(N, HEADS, HID)
    a_d0 = np.einsum("nhc,hc->nh", h0ad, adst0).astype(np.float32)  # [N, 4]
    for i, d in enumerate("fr"):
        shared[f"WihT_{d}"] = np.ascontiguousarray(inputs[f"Wih_{d}"].T).astype(ml_dtypes.bfloat16)
        shared[f"WhhT_{d}"] = np.ascontiguousarray(inputs[f"Whh_{d}"].T).astype(ml_dtypes.bfloat16)
    bsum = np.zeros((P, 48), np.float32)
    for i, d in enumerate("fr"):
        bs = (inputs[f"bih_{d}"] + inputs[f"bhh_{d}"]).astype(np.float32)  # [3072]
        bsum[:, i * 24 : (i + 1) * 24] = bs.reshape(24, P).T
    shared["bsum"] = bsum
    attw = np.zeros((P, 12), np.float32)
    aw = inputs["att_w"].astype(np.float32)
    attw[:, 0:6] = aw[0:HL].reshape(6, P).T
    attw[:, 6:12] = aw[HL:].reshape(6, P).T
    shared["attw"] = attw.astype(ml_dtypes.bfloat16)
    fcb = np.zeros((P, 8), np.float32)
    fcb[:, 0:4] = inputs["fc1_b"].reshape(4, P).T
    fcb[:, 4:8] = inputs["fc2_b"].reshape(4, P).T
    shared["fcb"] = fcb

    in_maps = []
    for k in range(NCORES):
        m = dict(shared)
        own_ad = np.zeros((NPCP, 4), np.float32)
        own_ad[0:NPC] = a_d0[k * NPC : (k + 1) * NPC]
        m["adtab0"] = np.ascontiguousarray(
            own_ad.reshape(NTILES_OWN, P, 4).transpose(1, 0, 2).reshape(P, 4 * NTILES_OWN)
        ).astype(ml_dtypes.bfloat16)
        m["srcidx"] = srcidx[k]
        m["dlt"] = dlt[k]
        m["stall"] = stall[k]
        m["sall"] = sall[k]
        m["poolmat"] = poolmat[k]
        in_maps.append(m)
    return in_maps


def get_kernel(nct):
    if nct not in _CACHE:
        nswdge = int(os.environ.get("KERNEL_NSWDGE", "1"))
        _CACHE[nct] = build_nc(nct, nswdge=nswdge)
    return _CACHE[nct]


def kernel(**inputs):
    nct, srcidx, dlt, poolmat, stall, sall = build_tables(inputs["edge_index"], inputs["batch"])
    nc = get_kernel(nct)
    in_maps = build_in_maps(inputs, nct, srcidx, dlt, poolmat, stall, sall)
    from concourse.bass_utils import run_bass_kernel_spmd

    res = run_bass_kernel_spmd(nc, in_maps, core_ids=list(range(NCORES)))
    out_T = res.results[0]["out_T"]
    return np.ascontiguousarray(out_T.T.astype(np.float32))
